# revision 21
# baseline (speedup 1.0000x reference)
"""Trainium2 Bass kernel for the nn_Experts MoE-LoRA problem.

Computes, for x = hidden_states.reshape(T, D):
    probs   = softmax(x @ Wr + br)
    w, idx  = top2(probs); combine[t,e] = w if e selected else 0
    base    = x @ W1                     (b1 folded into the gelu bias)
    t1      = einsum('td,erd->ter', x, A1)
    l1      = einsum('ter,efr->tef', t1, B1) * 2.0
    a       = gelu_tanh(base[:,None,:] + b1 + l1)
    ca      = a * combine[:,:,None]
    mix     = ca.sum(1)
    t2      = einsum('tef,erf->ter', ca, A2)
    l2      = einsum('ter,edr->td', t2, B2) * 2.0
    out     = mix @ W2 + combine.sum(-1,keepdims) * b2 + l2

Sharding: the F=8192 ff dimension is split across the 8 cores (Fs=1024
per core).  Each core holds the full token set and all 8 experts'
LoRA factors restricted to its F-slice, and produces a partial
out^T = W2s^T @ mix_s^T + l2_partial, which the host sums over cores.
The rank-1 `combine.sum * b2` term is added on the host (zero for the
reference's b2=0, but handled generally).

Top-2 sparsity via a gather-free SLOT decomposition: each token has
exactly 2 active experts (slot0 = argmax, slot1 = runner-up).  With
per-token 0/1 rank-row masks M_s (rank rows of the selected expert),
    z_s   = B1all^T (t1 (.) M_s) + base        (one full matmul per f,slot)
    ca_s  = gelu(z_s + b1) * w_s               (w_s = slot prob, bcast rows)
    mix   = ca_0 + ca_1
    t2c   = M_0 (.) (A2all^T ca_0) + M_1 (.) (A2all^T ca_1)
    l2    = B2all^T t2c
so the 8-expert dense path collapses to 2 slots: 4x less PE/ACT/DVE
work on the expert path.  Verified exact vs the dense reference.

On-chip layout is F-major ([F-slice, T]) so the F-contractions (A2,
W2) need no transposes.  The router runs as a 3-term bf16 split
(Wh@xh + Wh@xl + Wl@xh, fp32 psum accumulate) which matches fp32
top-2 selection with ~50x logit-gap margin while avoiding an 8MB fp32
x load.  All DRAM operands are host-staged so every DMA is contiguous
per partition.  W2-phase work for the first token chunk is interleaved
into the second chunk's main loop (PSUM budget: 2 base + 2 slot1 +
2 t2 + 2 out banks).
"""

import os
import sys

for _p in ("/opt/trn_rl_repo", os.path.join(os.path.dirname(os.path.abspath(__file__)))):
    if _p not in sys.path:
        sys.path.insert(0, _p)

import numpy as np
import ml_dtypes

import concourse.bass as bass
import concourse.mybir as mybir
import concourse.tile as tile
from concourse import bacc

BF16 = mybir.dt.bfloat16
F32 = mybir.dt.float32
AF = mybir.ActivationFunctionType
ALU = mybir.AluOpType
AX = mybir.AxisListType

E = 8      # experts
K = 2      # top-k
D = 2048   # hidden
F = 8192   # ff dim (full)
R = 16     # lora rank
ER = E * R           # 128 packed rank rows
SCALING = 2.0
NCORES = 8
FS = F // NCORES   # per-core ff slice = 1024
P = 128
TCH = 512          # token chunk (one PSUM bank of fp32)
NST = 18           # packed router rows: 8 slot0-ind, 8 slot1-ind, w0, w1


# --------------------------------------------------------------------------
# device program
# --------------------------------------------------------------------------

def build_nc(T: int) -> bass.Bass:
    """Build the single-core Bass program (same program for all 8 cores;
    per-core data differs)."""
    assert T % TCH == 0
    n_tch = T // TCH
    n_mt = T // P          # token tiles
    KT = D // P            # contraction tiles over D = 16

    nc = bacc.Bacc("TRN2", target_bir_lowering=False, debug=False,
                   num_devices=NCORES)

    # ---- DRAM parameters (per-core data); all host-staged so each DMA is
    # contiguous per partition ----
    cstage = nc.dram_tensor("cstage", [NST, T], BF16).ap()
    # x^T bf16 hi/lo, layout [p, (tch k t')]
    xhi = nc.dram_tensor("xhi", [P, KT * T], BF16, kind="ExternalInput").ap()
    xlo = nc.dram_tensor("xlo", [P, KT * T], BF16, kind="ExternalInput").ap()
    # W1 slice: rows (f p), cols (k c)
    w1s = nc.dram_tensor("w1s", [(FS // P) * P, KT * P], BF16,
                         kind="ExternalInput").ap()
    w2s = nc.dram_tensor("w2s", [FS, D], BF16, kind="ExternalInput").ap()
    a1s = nc.dram_tensor("a1s", [P, KT * P], BF16, kind="ExternalInput").ap()
    # B1all^T: [128 rank rows, FS]
    b1aT = nc.dram_tensor("b1aT", [ER, FS], BF16, kind="ExternalInput").ap()
    # A2all^T staged: [p, (f rank)]
    a2s = nc.dram_tensor("a2s", [P, (FS // P) * ER], BF16,
                         kind="ExternalInput").ap()
    b2cT = nc.dram_tensor("b2cT", [ER, D], BF16, kind="ExternalInput").ap()
    wrh = nc.dram_tensor("wrh", [P, KT * E], BF16, kind="ExternalInput").ap()
    wrl = nc.dram_tensor("wrl", [P, KT * E], BF16, kind="ExternalInput").ap()
    brv = nc.dram_tensor("brv", [1, E], F32, kind="ExternalInput").ap()
    b1sM = nc.dram_tensor("b1sM", [P, FS // P], F32, kind="ExternalInput").ap()
    idf = nc.dram_tensor("idf", [P, P], F32, kind="ExternalInput").ap()
    idb = nc.dram_tensor("idb", [P, P], BF16, kind="ExternalInput").ap()
    outT = nc.dram_tensor("outT", [D, T], F32, kind="ExternalOutput").ap()

    with tile.TileContext(nc) as tc:
        _emit(tc, T, n_tch, n_mt, KT,
              xhi, xlo, w1s, w2s, a1s, b1aT, a2s, b2cT, wrh, wrl, brv, b1sM,
              outT, cstage, idf, idb)
    nc.compile()
    return nc


def _emit(tc, T, n_tch, n_mt, KT,
          xhi, xlo, w1s, w2s, a1s, b1aT, a2s, b2cT, wrh, wrl, brv, b1sM,
          outT, cstage, idf, idb):
    nc = tc.nc
    from contextlib import ExitStack
    ctx = ExitStack()

    resid = ctx.enter_context(tc.tile_pool(name="resid", bufs=1))

    # ---- small router weights first (tiny DMAs) ----
    wrh_all = resid.tile([P, KT * E], BF16, name="wrh_all", tag="wrh_all")
    nc.sync.dma_start(wrh_all[:], wrh[:, :])
    wrl_all = resid.tile([P, KT * E], BF16, name="wrl_all", tag="wrl_all")
    nc.sync.dma_start(wrl_all[:], wrl[:, :])
    wrh_t = [wrh_all[:, k * E:(k + 1) * E] for k in range(KT)]
    wrl_t = [wrl_all[:, k * E:(k + 1) * E] for k in range(KT)]

    brv_t = resid.tile([1, E], F32, name="brv_t", tag="brv_t")
    nc.sync.dma_start(brv_t[:], brv[:, :])
    b1s_t = resid.tile([P, FS // P], F32, name="b1s_t", tag="b1s_t")
    nc.sync.dma_start(b1s_t[:], b1sM[:, :])

    ident = resid.tile([P, P], F32, name="ident", tag="ident")
    nc.sync.dma_start(ident[:], idf[:, :])
    ident_bf = resid.tile([P, P], BF16, name="ident_bf", tag="ident_bf")
    nc.sync.dma_start(ident_bf[:], idb[:, :])

    # ---- x hi resident (router term 1 + all of phase A/B); the DMAs are
    # emitted inside the router block interleaved with the x-lo loads ----
    xbf_all = resid.tile([P, KT * T], BF16, name="xbf_all", tag="xbf_all")

    def xb(k, tch):
        o = (tch * KT + k) * TCH
        return xbf_all[:, o:o + TCH]

    a1_all = resid.tile([P, KT * P], BF16, name="a1_all", tag="a1_all")
    nc.sync.dma_start(a1_all[:], a1s[:, :])
    a1_t = [a1_all[:, k * P:(k + 1) * P] for k in range(KT)]

    # packed router outputs [18, T]: slot indicator rows + slot weights
    mst = resid.tile([NST, T], BF16, name="mst", tag="mst")
    # rank-row masks and slot-weight broadcasts, per (slot, token chunk)
    # so chunk 0's masks don't wait on chunk 1's broadcasts
    M_t = [[resid.tile([P, TCH], BF16, name=f"M{s}_{tc}", tag=f"M{s}_{tc}")
            for tc in range(n_tch)] for s in range(2)]
    wsl_t = [[resid.tile([P, TCH], BF16, name=f"w{s}_{tc}", tag=f"w{s}_{tc}")
              for tc in range(n_tch)] for s in range(2)]

    # ---------------- router (3-term bf16 split, fp32 psum) ----------------
    # logits^T [E, T] accumulates Wh^T@xh + Wh^T@xl + Wl^T@xh + br; then
    # per-token-tile PE transposes to [128, E] for the free-dim softmax/top-2.
    with tc.tile_pool(name="router_sb", bufs=3) as rsb, \
         tc.tile_pool(name="router_xl", bufs=1) as rxl, \
         tc.tile_pool(name="router_ps", bufs=2, space="PSUM") as rps, \
         tc.tile_pool(name="tp_ps", bufs=2, space="PSUM") as tps:
        xlo_all = rxl.tile([P, KT * T], BF16, name="xlo_all", tag="xlo_all")
        # interleave hi/lo per token chunk, in 4-ktile pieces, so chunk-0
        # router terms start as soon as the first 0.5MB lands
        for tch in range(n_tch):
            for q in range(0, KT, 4):
                cs = slice((tch * KT + q) * TCH, (tch * KT + q + 4) * TCH)
                nc.sync.dma_start(xbf_all[:, cs], xhi[:, cs])
                nc.sync.dma_start(xlo_all[:, cs], xlo[:, cs])

        def xl(k, tch):
            o = (tch * KT + k) * TCH
            return xlo_all[:, o:o + TCH]

        ones_row = resid.tile([1, TCH], F32, name="ones_row", tag="ones_row")
        nc.vector.memset(ones_row[:], 1.0)
        lgT = resid.tile([E, T], F32, name="lgT", tag="lgT")
        for tch2 in range(n_tch):
            plg = rps.tile([E, TCH], F32, name="plg", tag="plg")
            for k in range(KT):
                nc.tensor.matmul(plg[:], wrh_t[k][:], xb(k, tch2),
                                 start=(k == 0), stop=False)
            for k in range(KT):
                nc.tensor.matmul(plg[:], wrh_t[k][:], xl(k, tch2),
                                 start=False, stop=False)
            for k in range(KT):
                nc.tensor.matmul(plg[:], wrl_t[k][:], xb(k, tch2),
                                 start=False, stop=False)
            nc.tensor.matmul(plg[:], brv_t[:], ones_row[:],
                             start=False, stop=True)
            nc.scalar.copy(lgT[:, tch2 * TCH:(tch2 + 1) * TCH], plg[:])

        for m in range(n_mt):
            pr = rps.tile([P, E], F32, name="pr", tag="pr")
            nc.tensor.transpose(pr[:], lgT[:, m * P:(m + 1) * P],
                                ident[:E, :E])

            # softmax over the 8 logits (free dim)
            negmax = rsb.tile([P, 1], F32, name="negmax", tag="negmax")
            nc.vector.tensor_reduce(negmax[:], pr[:], axis=AX.X, op=ALU.max,
                                    negate=True)
            pexp = rsb.tile([P, E], F32, name="pexp", tag="pexp")
            nc.scalar.activation(pexp[:], pr[:], AF.Exp, bias=negmax[:, 0:1],
                                 scale=1.0)
            ssum = rsb.tile([P, 1], F32, name="ssum", tag="ssum")
            nc.vector.tensor_reduce(ssum[:], pexp[:], axis=AX.X, op=ALU.add)
            rsum = rsb.tile([P, 1], F32, name="rsum", tag="rsum")
            nc.vector.reciprocal(rsum[:], ssum[:])
            probs = rsb.tile([P, E], F32, name="probs", tag="probs")
            nc.vector.tensor_scalar_mul(probs[:], pexp[:], rsum[:, 0:1])

            # packed [128, 18]: cols 0:8 slot0-ind, 8:16 slot1-ind,
            # 16 w0 (= top prob), 17 w1 (= 2nd prob)
            pk = rsb.tile([P, NST], F32, name="pk", tag="pk")
            nc.vector.tensor_reduce(pk[:, 16:17], probs[:], axis=AX.X,
                                    op=ALU.max)
            nc.vector.tensor_scalar(pk[:, 0:8], probs[:], pk[:, 16:17], None,
                                    op0=ALU.is_ge)
            pm = rsb.tile([P, E], F32, name="pm", tag="pm")
            # pm = probs - 2*slot0  (pushes the argmax below everything)
            nc.vector.scalar_tensor_tensor(pm[:], pk[:, 0:8], -2.0, probs[:],
                                           op0=ALU.mult, op1=ALU.add)
            nc.vector.tensor_reduce(pk[:, 17:18], pm[:], axis=AX.X,
                                    op=ALU.max)
            mask2 = rsb.tile([P, E], F32, name="mask2", tag="mask2")
            nc.vector.tensor_scalar(mask2[:], probs[:], pk[:, 17:18], None,
                                    op0=ALU.is_ge)
            nc.vector.tensor_tensor(pk[:, 8:16], mask2[:], pk[:, 0:8],
                                    op=ALU.subtract)

            # transpose [128, 18] -> [18, 128], store as bf16 columns of mst
            ptp = tps.tile([NST, P], F32, name="ptp", tag="ptp")
            nc.tensor.transpose(ptp[:], pk[:], ident[:])
            nc.scalar.copy(mst[:, m * P:(m + 1) * P], ptp[:])

    # ---------------- remaining resident loads ----------------
    # W1 fully resident (4MB): loaded once, reused by both token chunks
    w1p = ctx.enter_context(tc.tile_pool(name="w1_sb", bufs=1))
    n_fs = FS // P     # 8 f-tiles per core
    n_dm = D // P      # 16 output d-tiles
    w1_t = []
    for f in range(n_fs):
        t = w1p.tile([P, KT * P], BF16, name=f"w1_{f}", tag=f"w1_{f}")
        nc.sync.dma_start(t[:], w1s[f * P:(f + 1) * P, :])
        w1_t.append(t)

    b1a_t = resid.tile([ER, FS], BF16, name="b1a", tag="b1a")
    nc.sync.dma_start(b1a_t[:], b1aT[:, :])

    a2_all = resid.tile([P, (FS // P) * ER], BF16, name="a2_all",
                        tag="a2_all")
    nc.sync.dma_start(a2_all[:], a2s[:, :])
    a2_t = [a2_all[:, f * ER:(f + 1) * ER] for f in range(FS // P)]

    # broadcast the packed router rows: stage through DRAM (SBUF-source
    # partition-broadcast DMA is rejected; DRAM APs are linear).  Spread
    # across two engine queues, one round per token chunk, so chunk 0's
    # masks are ready as soon as its router m-tiles are done.
    bq = [nc.gpsimd, nc.scalar]
    for tcc in range(n_tch):
        tcs = slice(tcc * TCH, (tcc + 1) * TCH)
        nc.gpsimd.dma_start(cstage[:, tcs], mst[:, tcs])
        for s in range(2):
            for e in range(E):
                bq[(8 * s + e) % 2].dma_start(
                    M_t[s][tcc][R * e:R * e + R, :],
                    cstage[8 * s + e:8 * s + e + 1, tcs].to_broadcast([R, TCH]))
        for s in range(2):
            bq[s % 2].dma_start(wsl_t[s][tcc][:],
                                cstage[16 + s:17 + s, tcs].to_broadcast([P, TCH]))

    # ---------------- t1 = A1^T-contraction (packed 128 rank rows) -------
    t1un = resid.tile([P, T], BF16, name="t1un", tag="t1un")
    with tc.tile_pool(name="t1_ps", bufs=2, space="PSUM") as t1ps:
        for tch in range(n_tch):
            pt1 = t1ps.tile([P, TCH], F32, name="pt1", tag="pt1")
            for k in range(KT):
                nc.tensor.matmul(pt1[:],
                                 a1_t[k][:],
                                 xb(k, tch),
                                 start=(k == 0), stop=(k == KT - 1))
            nc.scalar.copy(t1un[:, tch * TCH:(tch + 1) * TCH], pt1[:])
    # slot-masked t1: mt1_s = t1un (.) M_s, per token chunk
    mt1_t = [[None] * n_tch for _ in range(2)]
    for tcc in range(n_tch):
        for s in range(2):
            t = resid.tile([P, TCH], BF16, name=f"mt1_{s}_{tcc}",
                           tag=f"mt1_{s}_{tcc}")
            nc.vector.tensor_mul(t[:], t1un[:, tcc * TCH:(tcc + 1) * TCH],
                                 M_t[s][tcc][:])
            mt1_t[s][tcc] = t

    # ---------------- main pipeline ----------------
    main = ctx.enter_context(tc.tile_pool(name="main_sb", bufs=3))
    mixp = ctx.enter_context(tc.tile_pool(name="mix_sb", bufs=2))
    w2p = ctx.enter_context(tc.tile_pool(name="w2_sb", bufs=1))
    outp = ctx.enter_context(tc.tile_pool(name="out_sb", bufs=3))

    w2_t = []
    b2c_t = []

    def load_phase_b_weights():
        for f in range(n_fs):
            t = w2p.tile([P, D], BF16, name=f"w2_{f}", tag=f"w2_{f}")
            nc.sync.dma_start(t[:], w2s[f * P:(f + 1) * P, :])
            w2_t.append(t)
        t = resid.tile([ER, D], BF16, name="b2c", tag="b2c")
        nc.sync.dma_start(t[:], b2cT[:, :])
        b2c_t.append(t)

    mix_all = [None] * (n_fs * n_tch)
    t2c_all = [None] * n_tch

    with tc.tile_pool(name="base_ps", bufs=2, space="PSUM") as pbp, \
         tc.tile_pool(name="sl1_ps", bufs=2, space="PSUM") as plp, \
         tc.tile_pool(name="t2_ps", bufs=1, space="PSUM") as pt2p, \
         tc.tile_pool(name="o_ps", bufs=2, space="PSUM") as pop:

        def emit_w2_dm(dm, tch):
            # out^T d-tile for one token chunk: 8 W2 + 1 B2 matmul
            po = pop.tile([P, TCH], F32, name="po", tag="po")
            for f in range(n_fs):
                nc.tensor.matmul(po[:],
                                 w2_t[f][:, dm * P:(dm + 1) * P],
                                 mix_all[tch * n_fs + f][:],
                                 start=(f == 0), stop=False,
                                 skip_group_check=True)
            nc.tensor.matmul(po[:],
                             b2c_t[0][:, dm * P:(dm + 1) * P],
                             t2c_all[tch][:],
                             start=False, stop=True,
                             skip_group_check=True)
            o_sb = outp.tile([P, TCH], F32, name="o_sb", tag="o_sb")
            nc.scalar.copy(o_sb[:], po[:])
            nc.sync.dma_start(
                outT[dm * P:(dm + 1) * P, tch * TCH:(tch + 1) * TCH],
                o_sb[:])

        for tch in range(n_tch):
            ts = slice(tch * TCH, (tch + 1) * TCH)

            pt2 = [pt2p.tile([P, TCH], F32, name=f"pt2_{s}", tag=f"pt2_{s}")
                   for s in range(2)]
            mix_t = [mixp.tile([P, TCH], BF16, name=f"mix{f}", tag=f"mix{f}")
                     for f in range(n_fs)]

            # t2 matmuls are emitted one f-iteration late so the PE never
            # stalls on the DVE chain that produces ca.
            pending_t2 = []

            def flush_t2():
                for (f0, s0, ca0) in pending_t2:
                    nc.tensor.matmul(pt2[s0][:], a2_t[f0][:], ca0[:],
                                     start=(f0 == 0), stop=(f0 == n_fs - 1),
                                     skip_group_check=True)
                pending_t2.clear()

            for f in range(n_fs):
                if tch == 0 and f == 4:
                    # W2/B2 are needed from chunk 1 on; loading mid-chunk-0
                    # keeps the startup DMA window free for x/W1/router
                    load_phase_b_weights()
                flush_t2()
                # base^T tile = W1s^T @ x^T   [128 f-rows, TCH tokens]
                pb = pbp.tile([P, TCH], F32, name="pb", tag="pb")
                for k in range(KT):
                    nc.tensor.matmul(pb[:],
                                     w1_t[f][:, k * P:(k + 1) * P],
                                     xb(k, tch),
                                     start=(k == 0), stop=False)
                # slot1 l1 into its own bank; z1 = base + l1_slot1 on DVE
                # (one psum + one sbuf operand) so the PE never re-streams
                # base through an identity matmul
                base_sb = main.tile([P, TCH], BF16, name="base_sb",
                                    tag="base_sb", bufs=2)
                nc.scalar.copy(base_sb[:], pb[:])
                pl = plp.tile([P, TCH], F32, name="pl", tag="pl")
                nc.tensor.matmul(pl[:], b1a_t[:, f * P:(f + 1) * P],
                                 mt1_t[1][tch][:],
                                 start=True, stop=True)
                z1_sb = main.tile([P, TCH], BF16, name="z1_sb",
                                  tag="z1_sb", bufs=2)
                nc.vector.tensor_add(z1_sb[:], pl[:], base_sb[:])
                # slot0: l1 accumulates into the base psum group
                nc.tensor.matmul(pb[:], b1a_t[:, f * P:(f + 1) * P],
                                 mt1_t[0][tch][:],
                                 start=False, stop=True)

                cas = []
                for s, ps in ((0, pb[:]), (1, z1_sb[:])):
                    # a = gelu_tanh(z + b1)
                    a_sb = main.tile([P, TCH], BF16, name="a_sb",
                                     tag=f"a_sb{s}", bufs=2)
                    nc.scalar.activation(a_sb[:], ps,
                                         AF.Gelu_apprx_tanh,
                                         bias=b1s_t[:, f:f + 1], scale=1.0)
                    # ca_s = a * w_s  (slot prob, broadcast rows)
                    ca = main.tile([P, TCH], BF16, name="ca_sb",
                                   tag=f"ca{s}", bufs=2)
                    nc.vector.tensor_mul(ca[:], a_sb[:], wsl_t[s][tch][:])
                    cas.append(ca)
                    pending_t2.append((f, s, ca))
                nc.vector.tensor_add(mix_t[f][:], cas[0][:], cas[1][:])

                # interleave previous chunk's W2 output work (2 d-tiles
                # per f-iteration) into this chunk's main loop
                if tch == 1:
                    emit_w2_dm(2 * f, 0)
                    emit_w2_dm(2 * f + 1, 0)
            flush_t2()

            # t2c = M0 (.) t2full_0 + M1 (.) t2full_1  (compact 128 ranks)
            tq = main.tile([P, TCH], BF16, name="tq", tag="tq", bufs=1)
            nc.vector.tensor_mul(tq[:], pt2[0][:], M_t[0][tch][:])
            tq2 = main.tile([P, TCH], BF16, name="tq2", tag="tq2", bufs=1)
            nc.vector.tensor_mul(tq2[:], pt2[1][:], M_t[1][tch][:])
            t2c = main.tile([P, TCH], BF16, name="t2c", tag=f"t2c_{tch}",
                            bufs=1)
            nc.vector.tensor_add(t2c[:], tq[:], tq2[:])
            t2c_all[tch] = t2c
            for f in range(n_fs):
                mix_all[tch * n_fs + f] = mix_t[f]

        # tail: W2 output work for the last token chunk
        for dm in range(n_dm):
            emit_w2_dm(dm, n_tch - 1)

    ctx.close()


# --------------------------------------------------------------------------
# host-side sharding / gather
# --------------------------------------------------------------------------

def make_in_maps(hidden_states, Wr, br, W1, b1, W2, b2, A1, B1, A2, B2):
    """Build the 8 per-core input dicts from full fp32 inputs."""
    hidden_states, Wr, br, W1, b1, W2, b2, A1, B1, A2, B2 = (
        np.asarray(a) for a in
        (hidden_states, Wr, br, W1, b1, W2, b2, A1, B1, A2, B2))
    bf16 = ml_dtypes.bfloat16
    T = hidden_states.shape[0] * hidden_states.shape[1]
    n_tch = T // TCH
    KT = D // P
    x = np.ascontiguousarray(hidden_states.reshape(T, D).astype(np.float32))
    xT = np.ascontiguousarray(x.T)                      # [D, T]
    xh32 = xT.astype(bf16).astype(np.float32)
    xl32 = xT - xh32

    def stage_x(a32):
        # [D, T] -> [p, (tch k t')]
        return np.ascontiguousarray(
            a32.reshape(KT, P, n_tch, TCH).transpose(1, 2, 0, 3)
            .reshape(P, KT * T)).astype(bf16)

    xhi = stage_x(xh32)
    xlo = stage_x(xl32)

    wr_h32 = Wr.astype(np.float32).astype(bf16).astype(np.float32)
    wr_l32 = Wr.astype(np.float32) - wr_h32

    def stage_wr(a32):
        # [D, E] -> [p, (k e)]
        return np.ascontiguousarray(
            a32.reshape(KT, P, E).transpose(1, 0, 2)
            .reshape(P, KT * E)).astype(bf16)

    wrh = stage_wr(wr_h32)
    wrl = stage_wr(wr_l32)
    brv = br.astype(np.float32).reshape(1, E)

    # a1: [p, (k r)] with r the 8*16 packed rank rows
    a1T = np.zeros((D, P), dtype=np.float32)
    for e in range(E):
        a1T[:, R * e:R * e + R] = A1[e].T                  # A1[e] is [R, D]
    a1s = np.ascontiguousarray(
        a1T.reshape(KT, P, P).transpose(1, 0, 2).reshape(P, KT * P)
    ).astype(bf16)

    n_fs = FS // P
    in_maps = []
    for c in range(NCORES):
        s = slice(c * FS, (c + 1) * FS)
        # W1 slice -> rows (f p), cols (k c)
        w1sl = np.ascontiguousarray(
            W1[:, s].reshape(KT, P, n_fs, P).transpose(2, 1, 0, 3)
            .reshape(n_fs * P, KT * P)).astype(bf16)
        w2sl = np.ascontiguousarray(W2[s, :]).astype(bf16)

        # B1all^T [128 ranks, FS]; A2all^T staged [p, (f rank)]
        b1a = np.zeros((ER, FS), dtype=bf16)
        a2aT = np.zeros((FS, ER), dtype=np.float32)
        b2c = np.zeros((ER, D), dtype=bf16)
        for e in range(E):
            b1a[R * e:R * e + R, :] = (B1[e, s, :].T * SCALING).astype(bf16)
            a2aT[:, R * e:R * e + R] = A2[e, :, s].T
            b2c[R * e:R * e + R, :] = (B2[e].T * SCALING).astype(bf16)
        a2st = np.ascontiguousarray(
            a2aT.reshape(n_fs, P, ER).transpose(1, 0, 2)
            .reshape(P, n_fs * ER)).astype(bf16)

        b1sM = np.ascontiguousarray(
            b1[s].astype(np.float32).reshape(n_fs, P).T)   # [P, FS//P]

        in_maps.append(dict(
            xhi=xhi, xlo=xlo, w1s=w1sl, w2s=w2sl, a1s=a1s,
            b1aT=b1a, a2s=a2st, b2cT=b2c, wrh=wrh, wrl=wrl, brv=brv,
            b1sM=b1sM,
            idf=np.eye(P, dtype=np.float32),
            idb=np.eye(P, dtype=np.float32).astype(bf16),
        ))
    return in_maps


_nc_cache = {}


def _get_nc(T):
    if T not in _nc_cache:
        _nc_cache[T] = build_nc(T)
    return _nc_cache[T]


_last_results = None


def _ensure_ntff_hook():
    """Install the axon NTFF profiling hook if the image's antenv lacks
    axon_hooks (needed for trace=True timing under axon)."""
    import types
    try:
        import antenv
        if "antenv.axon_hooks" not in sys.modules:
            mod = types.ModuleType("antenv.axon_hooks")
            mod._hook = None

            def set_axon_ntff_profile_hook(h):
                mod._hook = h

            def get_axon_ntff_profile_hook():
                return mod._hook

            mod.set_axon_ntff_profile_hook = set_axon_ntff_profile_hook
            mod.get_axon_ntff_profile_hook = get_axon_ntff_profile_hook
            sys.modules["antenv.axon_hooks"] = mod
            antenv.axon_hooks = mod
        hooks = sys.modules["antenv.axon_hooks"]
        if hooks.get_axon_ntff_profile_hook() is None:
            if "/root/.axon_site" not in sys.path:
                sys.path.insert(0, "/root/.axon_site")
            from trn_agent_boot.trn_boot import _ntff_profile_via_ctypes
            hooks.set_axon_ntff_profile_hook(
                _ntff_profile_via_ctypes("/opt/axon/libaxon_pjrt.so"))
    except Exception as e:  # profiling is best-effort
        print(f"ntff hook setup failed: {e}", file=sys.stderr)


def kernel(hidden_states, Wr, br, W1, b1, W2, b2, A1, B1, A2, B2,
           trace=False):
    global _last_results
    from concourse.bass_utils import run_bass_kernel_spmd
    if trace:
        _ensure_ntff_hook()

    B, S, _ = hidden_states.shape
    T = B * S
    nc = _get_nc(T)
    in_maps = make_in_maps(hidden_states, Wr, br, W1, b1, W2, b2,
                           A1, B1, A2, B2)
    tmpdir = os.environ.get("KERNEL_TRACE_DIR") or None
    if tmpdir:
        os.makedirs(tmpdir, exist_ok=True)
    res = run_bass_kernel_spmd(nc, in_maps, list(range(NCORES)), trace=trace,
                               tmpdir=tmpdir)
    _last_results = res
    out = np.zeros((T, D), dtype=np.float64)
    for c in range(NCORES):
        out += res.results[c]["outT"].astype(np.float64).T

    # rank-1 bias term: combine.sum(-1) * b2 (zero for the reference's b2=0,
    # computed host-side from the fp32 router for generality)
    b2f = np.asarray(b2, dtype=np.float64)
    if np.any(b2f):
        x = np.asarray(hidden_states, np.float32).reshape(T, D)
        logits = x @ np.asarray(Wr, np.float32) + np.asarray(br, np.float32)
        mx = logits.max(-1, keepdims=True)
        p = np.exp(logits - mx)
        p /= p.sum(-1, keepdims=True)
        csum = np.sort(p, axis=-1)[:, -K:].sum(-1)
        out += np.outer(csum.astype(np.float64), b2f)
    return out.astype(np.float32).reshape(B, S, D)


# revision 22
# speedup vs baseline: 1.0073x; 1.0073x over previous
"""Trainium2 Bass kernel for the nn_Experts MoE-LoRA problem.

Computes, for x = hidden_states.reshape(T, D):
    probs   = softmax(x @ Wr + br)
    w, idx  = top2(probs); combine[t,e] = w if e selected else 0
    base    = x @ W1                     (b1 folded into the gelu bias)
    t1      = einsum('td,erd->ter', x, A1)
    l1      = einsum('ter,efr->tef', t1, B1) * 2.0
    a       = gelu_tanh(base[:,None,:] + b1 + l1)
    ca      = a * combine[:,:,None]
    mix     = ca.sum(1)
    t2      = einsum('tef,erf->ter', ca, A2)
    l2      = einsum('ter,edr->td', t2, B2) * 2.0
    out     = mix @ W2 + combine.sum(-1,keepdims) * b2 + l2

Sharding: the F=8192 ff dimension is split across the 8 cores (Fs=1024
per core).  Each core holds the full token set and all 8 experts'
LoRA factors restricted to its F-slice, and produces a partial
out^T = W2s^T @ mix_s^T + l2_partial, which the host sums over cores.
The rank-1 `combine.sum * b2` term is added on the host (zero for the
reference's b2=0, but handled generally).

Top-2 sparsity via a gather-free SLOT decomposition: each token has
exactly 2 active experts (slot0 = argmax, slot1 = runner-up).  With
per-token 0/1 rank-row masks M_s (rank rows of the selected expert),
    z_s   = B1all^T (t1 (.) M_s) + base        (one full matmul per f,slot)
    ca_s  = gelu(z_s + b1) * w_s               (w_s = slot prob, bcast rows)
    mix   = ca_0 + ca_1
    t2c   = M_0 (.) (A2all^T ca_0) + M_1 (.) (A2all^T ca_1)
    l2    = B2all^T t2c
so the 8-expert dense path collapses to 2 slots: 4x less PE/ACT/DVE
work on the expert path.  Verified exact vs the dense reference.

On-chip layout is F-major ([F-slice, T]) so the F-contractions (A2,
W2) need no transposes.  The router runs as a 3-term bf16 split
(Wh@xh + Wh@xl + Wl@xh, fp32 psum accumulate) which matches fp32
top-2 selection with ~50x logit-gap margin while avoiding an 8MB fp32
x load.  All DRAM operands are host-staged so every DMA is contiguous
per partition.  W2-phase work for the first token chunk is interleaved
into the second chunk's main loop (PSUM budget: 2 base + 2 slot1 +
2 t2 + 2 out banks).
"""

import os
import sys

for _p in ("/opt/trn_rl_repo", os.path.join(os.path.dirname(os.path.abspath(__file__)))):
    if _p not in sys.path:
        sys.path.insert(0, _p)

import numpy as np
import ml_dtypes

import concourse.bass as bass
import concourse.mybir as mybir
import concourse.tile as tile
from concourse import bacc

BF16 = mybir.dt.bfloat16
F32 = mybir.dt.float32
AF = mybir.ActivationFunctionType
ALU = mybir.AluOpType
AX = mybir.AxisListType

E = 8      # experts
K = 2      # top-k
D = 2048   # hidden
F = 8192   # ff dim (full)
R = 16     # lora rank
ER = E * R           # 128 packed rank rows
SCALING = 2.0
NCORES = 8
FS = F // NCORES   # per-core ff slice = 1024
P = 128
TCH = 512          # token chunk (one PSUM bank of fp32)
NST = 18           # packed router rows: 8 slot0-ind, 8 slot1-ind, w0, w1


# --------------------------------------------------------------------------
# device program
# --------------------------------------------------------------------------

def build_nc(T: int) -> bass.Bass:
    """Build the single-core Bass program (same program for all 8 cores;
    per-core data differs)."""
    assert T % TCH == 0
    n_tch = T // TCH
    n_mt = T // P          # token tiles
    KT = D // P            # contraction tiles over D = 16

    nc = bacc.Bacc("TRN2", target_bir_lowering=False, debug=False,
                   num_devices=NCORES)

    # ---- DRAM parameters (per-core data); all host-staged so each DMA is
    # contiguous per partition ----
    cstage = nc.dram_tensor("cstage", [NST, T], BF16).ap()
    # x^T bf16 hi/lo, layout [p, (tch k t')]
    xhi = nc.dram_tensor("xhi", [P, KT * T], BF16, kind="ExternalInput").ap()
    xlo = nc.dram_tensor("xlo", [P, KT * T], BF16, kind="ExternalInput").ap()
    # W1 slice: rows (f p), cols (k c)
    w1s = nc.dram_tensor("w1s", [(FS // P) * P, KT * P], BF16,
                         kind="ExternalInput").ap()
    w2s = nc.dram_tensor("w2s", [FS, D], BF16, kind="ExternalInput").ap()
    a1s = nc.dram_tensor("a1s", [P, KT * P], BF16, kind="ExternalInput").ap()
    # B1all^T: [128 rank rows, FS]
    b1aT = nc.dram_tensor("b1aT", [ER, FS], BF16, kind="ExternalInput").ap()
    # A2all^T staged: [p, (f rank)]
    a2s = nc.dram_tensor("a2s", [P, (FS // P) * ER], BF16,
                         kind="ExternalInput").ap()
    b2cT = nc.dram_tensor("b2cT", [ER, D], BF16, kind="ExternalInput").ap()
    wrh = nc.dram_tensor("wrh", [P, KT * E], BF16, kind="ExternalInput").ap()
    wrl = nc.dram_tensor("wrl", [P, KT * E], BF16, kind="ExternalInput").ap()
    brv = nc.dram_tensor("brv", [1, E], F32, kind="ExternalInput").ap()
    b1sM = nc.dram_tensor("b1sM", [P, FS // P], F32, kind="ExternalInput").ap()
    idf = nc.dram_tensor("idf", [P, P], F32, kind="ExternalInput").ap()
    idb = nc.dram_tensor("idb", [P, P], BF16, kind="ExternalInput").ap()
    outT = nc.dram_tensor("outT", [D, T], F32, kind="ExternalOutput").ap()

    with tile.TileContext(nc) as tc:
        _emit(tc, T, n_tch, n_mt, KT,
              xhi, xlo, w1s, w2s, a1s, b1aT, a2s, b2cT, wrh, wrl, brv, b1sM,
              outT, cstage, idf, idb)
    nc.compile()
    return nc


def _emit(tc, T, n_tch, n_mt, KT,
          xhi, xlo, w1s, w2s, a1s, b1aT, a2s, b2cT, wrh, wrl, brv, b1sM,
          outT, cstage, idf, idb):
    nc = tc.nc
    from contextlib import ExitStack
    ctx = ExitStack()

    resid = ctx.enter_context(tc.tile_pool(name="resid", bufs=1))

    # ---- small router weights first (tiny DMAs) ----
    wrh_all = resid.tile([P, KT * E], BF16, name="wrh_all", tag="wrh_all")
    nc.sync.dma_start(wrh_all[:], wrh[:, :])
    wrl_all = resid.tile([P, KT * E], BF16, name="wrl_all", tag="wrl_all")
    nc.sync.dma_start(wrl_all[:], wrl[:, :])
    wrh_t = [wrh_all[:, k * E:(k + 1) * E] for k in range(KT)]
    wrl_t = [wrl_all[:, k * E:(k + 1) * E] for k in range(KT)]

    brv_t = resid.tile([1, E], F32, name="brv_t", tag="brv_t")
    nc.sync.dma_start(brv_t[:], brv[:, :])
    b1s_t = resid.tile([P, FS // P], F32, name="b1s_t", tag="b1s_t")
    nc.sync.dma_start(b1s_t[:], b1sM[:, :])

    ident = resid.tile([P, P], F32, name="ident", tag="ident")
    nc.sync.dma_start(ident[:], idf[:, :])
    ident_bf = resid.tile([P, P], BF16, name="ident_bf", tag="ident_bf")
    nc.sync.dma_start(ident_bf[:], idb[:, :])

    # ---- x hi resident (router term 1 + all of phase A/B); the DMAs are
    # emitted inside the router block interleaved with the x-lo loads ----
    xbf_all = resid.tile([P, KT * T], BF16, name="xbf_all", tag="xbf_all")

    def xb(k, tch):
        o = (tch * KT + k) * TCH
        return xbf_all[:, o:o + TCH]

    a1_all = resid.tile([P, KT * P], BF16, name="a1_all", tag="a1_all")
    nc.sync.dma_start(a1_all[:], a1s[:, :])
    a1_t = [a1_all[:, k * P:(k + 1) * P] for k in range(KT)]

    # packed router outputs [18, T]: slot indicator rows + slot weights
    mst = resid.tile([NST, T], BF16, name="mst", tag="mst")
    # rank-row masks and slot-weight broadcasts, per (slot, token chunk)
    # so chunk 0's masks don't wait on chunk 1's broadcasts
    M_t = [[resid.tile([P, TCH], BF16, name=f"M{s}_{tc}", tag=f"M{s}_{tc}")
            for tc in range(n_tch)] for s in range(2)]
    wsl_t = [[resid.tile([P, TCH], BF16, name=f"w{s}_{tc}", tag=f"w{s}_{tc}")
              for tc in range(n_tch)] for s in range(2)]

    # ---------------- router (3-term bf16 split, fp32 psum) ----------------
    # logits^T [E, T] accumulates Wh^T@xh + Wh^T@xl + Wl^T@xh + br; then
    # per-token-tile PE transposes to [128, E] for the free-dim softmax/top-2.
    with tc.tile_pool(name="router_sb", bufs=3) as rsb, \
         tc.tile_pool(name="router_xl", bufs=1) as rxl, \
         tc.tile_pool(name="router_ps", bufs=2, space="PSUM") as rps, \
         tc.tile_pool(name="tp_ps", bufs=2, space="PSUM") as tps:
        xlo_all = rxl.tile([P, KT * T], BF16, name="xlo_all", tag="xlo_all")
        # interleave hi/lo per token chunk, in 4-ktile pieces, so chunk-0
        # router terms start as soon as the first 0.5MB lands
        for tch in range(n_tch):
            for q in range(0, KT, 4):
                cs = slice((tch * KT + q) * TCH, (tch * KT + q + 4) * TCH)
                nc.sync.dma_start(xbf_all[:, cs], xhi[:, cs])
                nc.sync.dma_start(xlo_all[:, cs], xlo[:, cs])

        def xl(k, tch):
            o = (tch * KT + k) * TCH
            return xlo_all[:, o:o + TCH]

        ones_row = resid.tile([1, TCH], F32, name="ones_row", tag="ones_row")
        nc.vector.memset(ones_row[:], 1.0)
        lgT = resid.tile([E, T], F32, name="lgT", tag="lgT")
        for tch2 in range(n_tch):
            plg = rps.tile([E, TCH], F32, name="plg", tag="plg")
            for k in range(KT):
                nc.tensor.matmul(plg[:], wrh_t[k][:], xb(k, tch2),
                                 start=(k == 0), stop=False)
            for k in range(KT):
                nc.tensor.matmul(plg[:], wrh_t[k][:], xl(k, tch2),
                                 start=False, stop=False)
            for k in range(KT):
                nc.tensor.matmul(plg[:], wrl_t[k][:], xb(k, tch2),
                                 start=False, stop=False)
            nc.tensor.matmul(plg[:], brv_t[:], ones_row[:],
                             start=False, stop=True)
            nc.scalar.copy(lgT[:, tch2 * TCH:(tch2 + 1) * TCH], plg[:])

        for m in range(n_mt):
            pr = rps.tile([P, E], F32, name="pr", tag="pr")
            nc.tensor.transpose(pr[:], lgT[:, m * P:(m + 1) * P],
                                ident[:E, :E])

            # softmax over the 8 logits (free dim)
            negmax = rsb.tile([P, 1], F32, name="negmax", tag="negmax")
            nc.vector.tensor_reduce(negmax[:], pr[:], axis=AX.X, op=ALU.max,
                                    negate=True)
            pexp = rsb.tile([P, E], F32, name="pexp", tag="pexp")
            nc.scalar.activation(pexp[:], pr[:], AF.Exp, bias=negmax[:, 0:1],
                                 scale=1.0)
            ssum = rsb.tile([P, 1], F32, name="ssum", tag="ssum")
            nc.vector.tensor_reduce(ssum[:], pexp[:], axis=AX.X, op=ALU.add)
            rsum = rsb.tile([P, 1], F32, name="rsum", tag="rsum")
            nc.vector.reciprocal(rsum[:], ssum[:])
            probs = rsb.tile([P, E], F32, name="probs", tag="probs")
            nc.vector.tensor_scalar_mul(probs[:], pexp[:], rsum[:, 0:1])

            # packed [128, 18]: cols 0:8 slot0-ind, 8:16 slot1-ind,
            # 16 w0 (= top prob), 17 w1 (= 2nd prob)
            pk = rsb.tile([P, NST], F32, name="pk", tag="pk")
            nc.vector.tensor_reduce(pk[:, 16:17], probs[:], axis=AX.X,
                                    op=ALU.max)
            nc.vector.tensor_scalar(pk[:, 0:8], probs[:], pk[:, 16:17], None,
                                    op0=ALU.is_ge)
            pm = rsb.tile([P, E], F32, name="pm", tag="pm")
            # pm = probs - 2*slot0  (pushes the argmax below everything)
            nc.vector.scalar_tensor_tensor(pm[:], pk[:, 0:8], -2.0, probs[:],
                                           op0=ALU.mult, op1=ALU.add)
            nc.vector.tensor_reduce(pk[:, 17:18], pm[:], axis=AX.X,
                                    op=ALU.max)
            mask2 = rsb.tile([P, E], F32, name="mask2", tag="mask2")
            nc.vector.tensor_scalar(mask2[:], probs[:], pk[:, 17:18], None,
                                    op0=ALU.is_ge)
            nc.vector.tensor_tensor(pk[:, 8:16], mask2[:], pk[:, 0:8],
                                    op=ALU.subtract)

            # transpose [128, 18] -> [18, 128], store as bf16 columns of mst
            ptp = tps.tile([NST, P], F32, name="ptp", tag="ptp")
            nc.tensor.transpose(ptp[:], pk[:], ident[:])
            nc.scalar.copy(mst[:, m * P:(m + 1) * P], ptp[:])

    # ---------------- remaining resident loads ----------------
    # W1 fully resident (4MB): loaded once, reused by both token chunks
    w1p = ctx.enter_context(tc.tile_pool(name="w1_sb", bufs=1))
    n_fs = FS // P     # 8 f-tiles per core
    n_dm = D // P      # 16 output d-tiles
    w1_t = []
    for f in range(n_fs):
        t = w1p.tile([P, KT * P], BF16, name=f"w1_{f}", tag=f"w1_{f}")
        nc.sync.dma_start(t[:], w1s[f * P:(f + 1) * P, :])
        w1_t.append(t)

    b1a_t = resid.tile([ER, FS], BF16, name="b1a", tag="b1a")
    nc.sync.dma_start(b1a_t[:], b1aT[:, :])

    a2_all = resid.tile([P, (FS // P) * ER], BF16, name="a2_all",
                        tag="a2_all")
    nc.sync.dma_start(a2_all[:], a2s[:, :])
    a2_t = [a2_all[:, f * ER:(f + 1) * ER] for f in range(FS // P)]

    # broadcast the packed router rows: stage through DRAM (SBUF-source
    # partition-broadcast DMA is rejected; DRAM APs are linear).  Spread
    # across two engine queues, one round per token chunk, so chunk 0's
    # masks are ready as soon as its router m-tiles are done.
    bq = [nc.gpsimd, nc.scalar]
    for tcc in range(n_tch):
        tcs = slice(tcc * TCH, (tcc + 1) * TCH)
        nc.gpsimd.dma_start(cstage[:, tcs], mst[:, tcs])
        for s in range(2):
            for e in range(E):
                bq[(8 * s + e) % 2].dma_start(
                    M_t[s][tcc][R * e:R * e + R, :],
                    cstage[8 * s + e:8 * s + e + 1, tcs].to_broadcast([R, TCH]))
        for s in range(2):
            bq[s % 2].dma_start(wsl_t[s][tcc][:],
                                cstage[16 + s:17 + s, tcs].to_broadcast([P, TCH]))

    # ---------------- t1 = A1^T-contraction (packed 128 rank rows) -------
    t1un = resid.tile([P, T], BF16, name="t1un", tag="t1un")
    with tc.tile_pool(name="t1_ps", bufs=2, space="PSUM") as t1ps:
        for tch in range(n_tch):
            pt1 = t1ps.tile([P, TCH], F32, name="pt1", tag="pt1")
            for k in range(KT):
                nc.tensor.matmul(pt1[:],
                                 a1_t[k][:],
                                 xb(k, tch),
                                 start=(k == 0), stop=(k == KT - 1))
            nc.scalar.copy(t1un[:, tch * TCH:(tch + 1) * TCH], pt1[:])
    # slot-masked t1: mt1_s = t1un (.) M_s, per token chunk
    mt1_t = [[None] * n_tch for _ in range(2)]
    for tcc in range(n_tch):
        for s in range(2):
            t = resid.tile([P, TCH], BF16, name=f"mt1_{s}_{tcc}",
                           tag=f"mt1_{s}_{tcc}")
            nc.vector.tensor_mul(t[:], t1un[:, tcc * TCH:(tcc + 1) * TCH],
                                 M_t[s][tcc][:])
            mt1_t[s][tcc] = t

    # ---------------- main pipeline ----------------
    main = ctx.enter_context(tc.tile_pool(name="main_sb", bufs=3))
    mixp = ctx.enter_context(tc.tile_pool(name="mix_sb", bufs=2))
    w2p = ctx.enter_context(tc.tile_pool(name="w2_sb", bufs=1))
    outp = ctx.enter_context(tc.tile_pool(name="out_sb", bufs=3))

    w2_t = []
    b2c_t = []

    def load_phase_b_weights():
        for f in range(n_fs):
            t = w2p.tile([P, D], BF16, name=f"w2_{f}", tag=f"w2_{f}")
            nc.sync.dma_start(t[:], w2s[f * P:(f + 1) * P, :])
            w2_t.append(t)
        t = resid.tile([ER, D], BF16, name="b2c", tag="b2c")
        nc.sync.dma_start(t[:], b2cT[:, :])
        b2c_t.append(t)

    mix_all = [None] * (n_fs * n_tch)
    t2c_all = [None] * n_tch

    with tc.tile_pool(name="base_ps", bufs=3, space="PSUM") as pbp, \
         tc.tile_pool(name="sl1_ps", bufs=1, space="PSUM") as plp, \
         tc.tile_pool(name="t2_ps", bufs=1, space="PSUM") as pt2p, \
         tc.tile_pool(name="o_ps", bufs=2, space="PSUM") as pop:

        def emit_w2_dm(dm, tch):
            # out^T d-tile for one token chunk: 8 W2 + 1 B2 matmul
            po = pop.tile([P, TCH], F32, name="po", tag="po")
            for f in range(n_fs):
                nc.tensor.matmul(po[:],
                                 w2_t[f][:, dm * P:(dm + 1) * P],
                                 mix_all[tch * n_fs + f][:],
                                 start=(f == 0), stop=False,
                                 skip_group_check=True)
            nc.tensor.matmul(po[:],
                             b2c_t[0][:, dm * P:(dm + 1) * P],
                             t2c_all[tch][:],
                             start=False, stop=True,
                             skip_group_check=True)
            o_sb = outp.tile([P, TCH], F32, name="o_sb", tag="o_sb")
            nc.scalar.copy(o_sb[:], po[:])
            nc.sync.dma_start(
                outT[dm * P:(dm + 1) * P, tch * TCH:(tch + 1) * TCH],
                o_sb[:])

        for tch in range(n_tch):
            ts = slice(tch * TCH, (tch + 1) * TCH)

            pt2 = [pt2p.tile([P, TCH], F32, name=f"pt2_{s}", tag=f"pt2_{s}")
                   for s in range(2)]
            mix_t = [mixp.tile([P, TCH], BF16, name=f"mix{f}", tag=f"mix{f}")
                     for f in range(n_fs)]

            # t2 matmuls are emitted one f-iteration late so the PE never
            # stalls on the DVE chain that produces ca.
            pending_t2 = []

            def flush_t2():
                for (f0, s0, ca0) in pending_t2:
                    nc.tensor.matmul(pt2[s0][:], a2_t[f0][:], ca0[:],
                                     start=(f0 == 0), stop=(f0 == n_fs - 1),
                                     skip_group_check=True)
                pending_t2.clear()

            for f in range(n_fs):
                if tch == 0 and f == 4:
                    # W2/B2 are needed from chunk 1 on; loading mid-chunk-0
                    # keeps the startup DMA window free for x/W1/router
                    load_phase_b_weights()
                flush_t2()
                # base^T tile = W1s^T @ x^T   [128 f-rows, TCH tokens]
                pb = pbp.tile([P, TCH], F32, name="pb", tag="pb")
                for k in range(KT):
                    nc.tensor.matmul(pb[:],
                                     w1_t[f][:, k * P:(k + 1) * P],
                                     xb(k, tch),
                                     start=(k == 0), stop=False)
                # slot1 l1 into its own bank; z1 = base + l1_slot1 on DVE
                # (one psum + one sbuf operand) so the PE never re-streams
                # base through an identity matmul
                base_sb = main.tile([P, TCH], BF16, name="base_sb",
                                    tag="base_sb", bufs=2)
                nc.scalar.copy(base_sb[:], pb[:])
                pl = plp.tile([P, TCH], F32, name="pl", tag="pl")
                nc.tensor.matmul(pl[:], b1a_t[:, f * P:(f + 1) * P],
                                 mt1_t[1][tch][:],
                                 start=True, stop=True)
                z1_sb = main.tile([P, TCH], BF16, name="z1_sb",
                                  tag="z1_sb", bufs=2)
                nc.vector.tensor_add(z1_sb[:], pl[:], base_sb[:])
                # slot0: l1 accumulates into the base psum group
                nc.tensor.matmul(pb[:], b1a_t[:, f * P:(f + 1) * P],
                                 mt1_t[0][tch][:],
                                 start=False, stop=True)

                cas = []
                for s, ps in ((0, pb[:]), (1, z1_sb[:])):
                    # a = gelu_tanh(z + b1)
                    a_sb = main.tile([P, TCH], BF16, name="a_sb",
                                     tag=f"a_sb{s}", bufs=2)
                    nc.scalar.activation(a_sb[:], ps,
                                         AF.Gelu_apprx_tanh,
                                         bias=b1s_t[:, f:f + 1], scale=1.0)
                    # ca_s = a * w_s  (slot prob, broadcast rows)
                    ca = main.tile([P, TCH], BF16, name="ca_sb",
                                   tag=f"ca{s}", bufs=2)
                    nc.vector.tensor_mul(ca[:], a_sb[:], wsl_t[s][tch][:])
                    cas.append(ca)
                    pending_t2.append((f, s, ca))
                nc.vector.tensor_add(mix_t[f][:], cas[0][:], cas[1][:])

                # interleave previous chunk's W2 output work (2 d-tiles
                # per f-iteration) into this chunk's main loop
                if tch == 1:
                    emit_w2_dm(2 * f, 0)
                    emit_w2_dm(2 * f + 1, 0)
            flush_t2()

            # t2c = M0 (.) t2full_0 + M1 (.) t2full_1  (compact 128 ranks)
            tq = main.tile([P, TCH], BF16, name="tq", tag="tq", bufs=1)
            nc.vector.tensor_mul(tq[:], pt2[0][:], M_t[0][tch][:])
            tq2 = main.tile([P, TCH], BF16, name="tq2", tag="tq2", bufs=1)
            nc.vector.tensor_mul(tq2[:], pt2[1][:], M_t[1][tch][:])
            t2c = main.tile([P, TCH], BF16, name="t2c", tag=f"t2c_{tch}",
                            bufs=1)
            nc.vector.tensor_add(t2c[:], tq[:], tq2[:])
            t2c_all[tch] = t2c
            for f in range(n_fs):
                mix_all[tch * n_fs + f] = mix_t[f]

        # tail: W2 output work for the last token chunk
        for dm in range(n_dm):
            emit_w2_dm(dm, n_tch - 1)

    ctx.close()


# --------------------------------------------------------------------------
# host-side sharding / gather
# --------------------------------------------------------------------------

def make_in_maps(hidden_states, Wr, br, W1, b1, W2, b2, A1, B1, A2, B2):
    """Build the 8 per-core input dicts from full fp32 inputs."""
    hidden_states, Wr, br, W1, b1, W2, b2, A1, B1, A2, B2 = (
        np.asarray(a) for a in
        (hidden_states, Wr, br, W1, b1, W2, b2, A1, B1, A2, B2))
    bf16 = ml_dtypes.bfloat16
    T = hidden_states.shape[0] * hidden_states.shape[1]
    n_tch = T // TCH
    KT = D // P
    x = np.ascontiguousarray(hidden_states.reshape(T, D).astype(np.float32))
    xT = np.ascontiguousarray(x.T)                      # [D, T]
    xh32 = xT.astype(bf16).astype(np.float32)
    xl32 = xT - xh32

    def stage_x(a32):
        # [D, T] -> [p, (tch k t')]
        return np.ascontiguousarray(
            a32.reshape(KT, P, n_tch, TCH).transpose(1, 2, 0, 3)
            .reshape(P, KT * T)).astype(bf16)

    xhi = stage_x(xh32)
    xlo = stage_x(xl32)

    wr_h32 = Wr.astype(np.float32).astype(bf16).astype(np.float32)
    wr_l32 = Wr.astype(np.float32) - wr_h32

    def stage_wr(a32):
        # [D, E] -> [p, (k e)]
        return np.ascontiguousarray(
            a32.reshape(KT, P, E).transpose(1, 0, 2)
            .reshape(P, KT * E)).astype(bf16)

    wrh = stage_wr(wr_h32)
    wrl = stage_wr(wr_l32)
    brv = br.astype(np.float32).reshape(1, E)

    # a1: [p, (k r)] with r the 8*16 packed rank rows
    a1T = np.zeros((D, P), dtype=np.float32)
    for e in range(E):
        a1T[:, R * e:R * e + R] = A1[e].T                  # A1[e] is [R, D]
    a1s = np.ascontiguousarray(
        a1T.reshape(KT, P, P).transpose(1, 0, 2).reshape(P, KT * P)
    ).astype(bf16)

    n_fs = FS // P
    in_maps = []
    for c in range(NCORES):
        s = slice(c * FS, (c + 1) * FS)
        # W1 slice -> rows (f p), cols (k c)
        w1sl = np.ascontiguousarray(
            W1[:, s].reshape(KT, P, n_fs, P).transpose(2, 1, 0, 3)
            .reshape(n_fs * P, KT * P)).astype(bf16)
        w2sl = np.ascontiguousarray(W2[s, :]).astype(bf16)

        # B1all^T [128 ranks, FS]; A2all^T staged [p, (f rank)]
        b1a = np.zeros((ER, FS), dtype=bf16)
        a2aT = np.zeros((FS, ER), dtype=np.float32)
        b2c = np.zeros((ER, D), dtype=bf16)
        for e in range(E):
            b1a[R * e:R * e + R, :] = (B1[e, s, :].T * SCALING).astype(bf16)
            a2aT[:, R * e:R * e + R] = A2[e, :, s].T
            b2c[R * e:R * e + R, :] = (B2[e].T * SCALING).astype(bf16)
        a2st = np.ascontiguousarray(
            a2aT.reshape(n_fs, P, ER).transpose(1, 0, 2)
            .reshape(P, n_fs * ER)).astype(bf16)

        b1sM = np.ascontiguousarray(
            b1[s].astype(np.float32).reshape(n_fs, P).T)   # [P, FS//P]

        in_maps.append(dict(
            xhi=xhi, xlo=xlo, w1s=w1sl, w2s=w2sl, a1s=a1s,
            b1aT=b1a, a2s=a2st, b2cT=b2c, wrh=wrh, wrl=wrl, brv=brv,
            b1sM=b1sM,
            idf=np.eye(P, dtype=np.float32),
            idb=np.eye(P, dtype=np.float32).astype(bf16),
        ))
    return in_maps


_nc_cache = {}


def _get_nc(T):
    if T not in _nc_cache:
        _nc_cache[T] = build_nc(T)
    return _nc_cache[T]


_last_results = None


def _ensure_ntff_hook():
    """Install the axon NTFF profiling hook if the image's antenv lacks
    axon_hooks (needed for trace=True timing under axon)."""
    import types
    try:
        import antenv
        if "antenv.axon_hooks" not in sys.modules:
            mod = types.ModuleType("antenv.axon_hooks")
            mod._hook = None

            def set_axon_ntff_profile_hook(h):
                mod._hook = h

            def get_axon_ntff_profile_hook():
                return mod._hook

            mod.set_axon_ntff_profile_hook = set_axon_ntff_profile_hook
            mod.get_axon_ntff_profile_hook = get_axon_ntff_profile_hook
            sys.modules["antenv.axon_hooks"] = mod
            antenv.axon_hooks = mod
        hooks = sys.modules["antenv.axon_hooks"]
        if hooks.get_axon_ntff_profile_hook() is None:
            if "/root/.axon_site" not in sys.path:
                sys.path.insert(0, "/root/.axon_site")
            from trn_agent_boot.trn_boot import _ntff_profile_via_ctypes
            hooks.set_axon_ntff_profile_hook(
                _ntff_profile_via_ctypes("/opt/axon/libaxon_pjrt.so"))
    except Exception as e:  # profiling is best-effort
        print(f"ntff hook setup failed: {e}", file=sys.stderr)


def kernel(hidden_states, Wr, br, W1, b1, W2, b2, A1, B1, A2, B2,
           trace=False):
    global _last_results
    from concourse.bass_utils import run_bass_kernel_spmd
    if trace:
        _ensure_ntff_hook()

    B, S, _ = hidden_states.shape
    T = B * S
    nc = _get_nc(T)
    in_maps = make_in_maps(hidden_states, Wr, br, W1, b1, W2, b2,
                           A1, B1, A2, B2)
    tmpdir = os.environ.get("KERNEL_TRACE_DIR") or None
    if tmpdir:
        os.makedirs(tmpdir, exist_ok=True)
    res = run_bass_kernel_spmd(nc, in_maps, list(range(NCORES)), trace=trace,
                               tmpdir=tmpdir)
    _last_results = res
    out = np.zeros((T, D), dtype=np.float64)
    for c in range(NCORES):
        out += res.results[c]["outT"].astype(np.float64).T

    # rank-1 bias term: combine.sum(-1) * b2 (zero for the reference's b2=0,
    # computed host-side from the fp32 router for generality)
    b2f = np.asarray(b2, dtype=np.float64)
    if np.any(b2f):
        x = np.asarray(hidden_states, np.float32).reshape(T, D)
        logits = x @ np.asarray(Wr, np.float32) + np.asarray(br, np.float32)
        mx = logits.max(-1, keepdims=True)
        p = np.exp(logits - mx)
        p /= p.sum(-1, keepdims=True)
        csum = np.sort(p, axis=-1)[:, -K:].sum(-1)
        out += np.outer(csum.astype(np.float64), b2f)
    return out.astype(np.float32).reshape(B, S, D)
